# revision 12
# baseline (speedup 1.0000x reference)
"""Trainium2 Bass kernel: fp8-quantized Dense (8192x4096 @ 4096x16384) + bias + tanh-GELU.

Strategy (tensor-parallel over units, 8 cores), v4 "wide prologue":
  - host: transpose x -> xT [d_in, tokens]; shard kernel/bias along units.
  - device per core:
      phase 1: amax scan of the kernel shard spread over FOUR dma trigger
               queues (sync/scalar/tensor/gpsimd) with 6-deep staging so the
               wire stays saturated; AllReduce(max) for k issues the moment
               the k reduce tree completes.  Then the x-slice scan on three
               queues (gpsimd sits behind the k collective), CC_x at its end.
               The first 8 k slabs are retained f32 in the idle xq-pool
               slots so only 24 slabs need re-streaming.
      phase 2: xb0 (block-0 tokens) prestages f32, then the k re-stream
               starts immediately -- while CC_x is still in flight -- so by
               the time the x scale lands most of the kernel shard is
               already in SBUF.  kq quantizes on ACT in ko order (retained
               slabs first, re-streamed as they arrive).
      phase 3: identical to v3: block 0 runs kk-outer over two halves of 8
               psum banks consuming kq slabs as they are produced; later
               blocks are ub-outer/kk-inner.  Epilogue fuses
               gelu_tanh(psum*inv_scale + bias) on ACT, written as fp16.
  - fp8 numerics: unchanged from v3 -- quantize with the correctly rounded
    s = RNE(224/amax) (Newton + candidate selection), TRN fp8e4 matches
    OCP e4m3fn exactly inside [-240, 240], dequant scale amax_x*amax_k/224^2
    restores the reference computation.
  - output is produced transposed ([units, tokens] per core) in fp16; the
    host gathers shards and returns the [tokens, units] f32 view.
"""

import sys

sys.path.insert(0, "/opt/trn_rl_repo")

from contextlib import ExitStack

import numpy as np

import concourse.bacc as bacc
import concourse.tile as tile
from concourse import mybir
from concourse.bass_utils import run_bass_kernel_spmd

P = 128
FP8_HW_MAX = 224.0  # 448/2: keeps hw fp8 values inside TRN's +/-240 range

TOKENS, D_IN, UNITS, N_CORES = 8192, 4096, 16384, 8

B0 = 256  # first token block: fully f32-staged so its loads never stall


def _blocks(tokens, tblk):
    """Token-block schedule: small warmup blocks so PE starts earlier."""
    assert tokens >= 2 * tblk and tblk >= 512
    head = [B0, B0]
    rest = tokens - sum(head)
    assert rest % tblk == 0
    return head + [tblk] * (rest // tblk)


def build(tokens=TOKENS, d_in=D_IN, units=UNITS, n_cores=N_CORES, tblk=512, nfree=512):
    us = units // n_cores
    ko_n = d_in // P          # 128-row f32 slabs along d_in
    kk_n = d_in // (2 * P)    # DoubleRow (256-contraction) steps
    nu = us // P              # 128-unit output blocks
    amx_t = tokens // n_cores # columns of xT this core amax-scans
    blocks = _blocks(tokens, tblk)

    assert d_in % (2 * P) == 0 and us % P == 0
    assert all(b % nfree == 0 or b in (B0, 2 * B0) for b in blocks)
    assert blocks[0] == B0 and nu == 16

    dt = mybir.dt
    f32 = dt.float32
    f16 = dt.float16
    fp8 = dt.float8e4
    X = mybir.AxisListType.X
    MAX = mybir.AluOpType.max
    COPY = mybir.ActivationFunctionType.Copy

    nc = bacc.Bacc("TRN2", target_bir_lowering=False, debug=False, num_devices=n_cores)
    xT = nc.dram_tensor("xT", [d_in, tokens], f32, kind="ExternalInput").ap()
    xsl = nc.dram_tensor("xsl", [d_in, amx_t], f32, kind="ExternalInput").ap()
    ksh = nc.dram_tensor("ksh", [d_in, us], f32, kind="ExternalInput").ap()
    bsh = nc.dram_tensor("bsh", [us], f32, kind="ExternalInput").ap()
    out = nc.dram_tensor("out", [us, tokens], f16, kind="ExternalOutput").ap()

    # Prologue trigger queues.  SP(sync) and Activation(scalar) have HWDGE
    # queues; gpsimd(Pool) can also trigger DMAs and its sequencer is idle
    # during the scan (the collectives only briefly occupy it), adding
    # descriptor-issue parallelism for the latency-bound scan pipeline.
    kscan_qs = (nc.sync, nc.scalar, nc.gpsimd)
    xscan_qs = (nc.sync, nc.scalar, nc.gpsimd)
    xb0_qs = (nc.sync, nc.scalar, nc.gpsimd)
    restream_qs = (nc.sync, nc.scalar, nc.gpsimd)

    def xload_q(i):
        return (nc.sync, nc.scalar)[i % 2]

    from concourse.tile_rust import add_dep_helper

    def chain_q(last_by_q, dma, q, reason):
        """Order DMA phases per queue on the wire: the DMA engines pull from
        all queue rings concurrently, so without explicit deps a later phase
        steals bandwidth from an earlier one it must not delay."""
        prev = last_by_q.get(id(q))
        if prev is not None:
            add_dep_helper(dma.ins, prev.ins, sync=True, reason=reason)
        last_by_q[id(q)] = dma

    # Per-partition SBUF budget (~207 KiB usable): kqp 64 + xqp 2x16=32 +
    # xb0p 32 + outp 5 + kstage 7x8=56 + xstage 4x4=16 + const/small ~1.
    with tile.TileContext(nc) as tc, ExitStack() as ctx:
        const = ctx.enter_context(tc.tile_pool(name="const", bufs=1))
        kstage = ctx.enter_context(tc.tile_pool(name="kstage", bufs=7))
        xstage = ctx.enter_context(tc.tile_pool(name="xstage", bufs=4))
        kqp = ctx.enter_context(tc.tile_pool(name="kqp", bufs=1))
        xqp = ctx.enter_context(tc.tile_pool(name="xqp", bufs=2))
        xb0p = ctx.enter_context(tc.tile_pool(name="xb0p", bufs=1))
        outp = ctx.enter_context(tc.tile_pool(name="outp", bufs=5))
        psum = ctx.enter_context(tc.tile_pool(name="psum", bufs=8, space="PSUM"))
        dram = ctx.enter_context(tc.tile_pool(name="dram", bufs=1, space="DRAM"))
        small = ctx.enter_context(tc.tile_pool(name="small", bufs=1))

        from concourse import bass_isa

        def partition_amax_to(dst, racc, name):
            """[P, ko_n] per-partition maxes -> [1,1] scalar in dst (SBUF)."""
            col = small.tile([P, 1], f32, name=f"{name}_col")
            nc.vector.tensor_reduce(col[:], racc[:], axis=X, op=MAX)
            nc.gpsimd.partition_all_reduce(col[:], col[:], P, bass_isa.ReduceOp.max)
            nc.vector.tensor_copy(dst, col[0:1, :])

        def allreduce_max_issue(src11, name):
            """Issue AllReduce(max) of a [1,1] scalar; returns the shared
            dram tile holding the result (read back separately)."""
            cc_in = dram.tile([1, 8], f32, name=f"{name}_in")
            z8 = small.tile([1, 8], f32, name=f"{name}_z8")
            nc.vector.memset(z8[:], 0.0)
            nc.vector.tensor_copy(z8[:, 0:1], src11)
            nc.sync.dma_start(cc_in[:], z8[:])
            cc_out = dram.tile([1, 8], f32, name=f"{name}_out", addr_space="Shared")
            nc.gpsimd.collective_compute(
                "AllReduce", MAX,
                replica_groups=[list(range(n_cores))],
                ins=[cc_in[:].opt()], outs=[cc_out[:].opt()],
            )
            return cc_out

        def bcast_scalar(src11, name):
            """[1,1] SBUF scalar (partition 0) -> [P,1] SBUF broadcast tile."""
            b = const.tile([P, 1], f32, name=f"{name}_b")
            nc.gpsimd.partition_broadcast(b, src11)
            return b

        # Correctly-rounded s = RNE(224/d): the quantize grid must bit-match
        # the reference's RNE(448/d)/2. DVE has no divide, and a 1-2 ulp-off
        # scale shifts the whole fp8 grid (~2.4e-3 rel err). Newton-refine
        # 224*recip(d) with a Dekker-exact residual, then pick among 5
        # float-constructed neighbor candidates the one minimizing |q*d-224|.
        NCAND = 5
        u32 = dt.uint32
        MUL = mybir.AluOpType.mult
        SUB = mybir.AluOpType.subtract
        ADD = mybir.AluOpType.add

        def tt(o, a, bb, op):
            nc.vector.tensor_tensor(o, a, bb, op)

        def exact_scale(g8, name):
            """g8: [1,8] SBUF allreduce result (slot 0 = amax).
            Returns ([1,1] scale s = RNE(224/max(amax,1e-12)), [1,1] d)."""
            d1 = small.tile([1, 1], f32, name=f"{name}_d1")
            nc.vector.tensor_scalar_max(d1[:], g8[:, 0:1], 1e-12)

            def c3(nm):
                return small.tile([1, 1, NCAND], f32, name=f"{name}_{nm}")

            def vsplit(src, pref, shape=(1, 1)):
                t_ = small.tile(list(shape), f32, name=f"{name}_{pref}_t")
                nc.vector.tensor_scalar_mul(t_[:], src, 4097.0)
                a_ = small.tile(list(shape), f32, name=f"{name}_{pref}_a")
                tt(a_[:], t_[:], src, SUB)
                hi = small.tile(list(shape), f32, name=f"{name}_{pref}_hi")
                tt(hi[:], t_[:], a_[:], SUB)
                lo = small.tile(list(shape), f32, name=f"{name}_{pref}_lo")
                tt(lo[:], src, hi[:], SUB)
                return hi, lo

            dh, dl = vsplit(d1[:], "dsp")

            def resid(qap, nm, shape, dhb, dlb, db):
                """exact q*d - 224 via Dekker two-product (f32 ops only)"""
                p_ = small.tile(list(shape), f32, name=f"{name}_{nm}_p")
                tt(p_[:], qap, db, MUL)
                qh, ql = vsplit(qap, f"{nm}_qs", shape)
                w = small.tile(list(shape), f32, name=f"{name}_{nm}_w")
                tt(w[:], qh[:], dhb, MUL)
                tt(w[:], w[:], p_[:], SUB)
                w2 = small.tile(list(shape), f32, name=f"{name}_{nm}_w2")
                tt(w2[:], qh[:], dlb, MUL)
                tt(w[:], w[:], w2[:], ADD)
                tt(w2[:], ql[:], dhb, MUL)
                tt(w[:], w[:], w2[:], ADD)
                tt(w2[:], ql[:], dlb, MUL)
                tt(w[:], w[:], w2[:], ADD)
                nc.vector.tensor_scalar_sub(p_[:], p_[:], FP8_HW_MAX)
                R_ = small.tile(list(shape), f32, name=f"{name}_{nm}_R")
                tt(R_[:], p_[:], w[:], ADD)
                return R_

            r1 = small.tile([1, 1], f32, name=f"{name}_r1")
            nc.vector.reciprocal(r1[:], d1[:])
            y0 = small.tile([1, 1], f32, name=f"{name}_y0")
            nc.vector.tensor_scalar_mul(y0[:], r1[:], FP8_HW_MAX)
            R0 = resid(y0[:], "n0", (1, 1), dh[:], dl[:], d1[:])
            corr = small.tile([1, 1], f32, name=f"{name}_corr")
            tt(corr[:], R0[:], r1[:], MUL)
            y = small.tile([1, 1], f32, name=f"{name}_y")
            tt(y[:], y0[:], corr[:], SUB)

            um = small.tile([1, 1], f32, name=f"{name}_um")
            nc.vector.tensor_scalar(
                um[:].bitcast(u32), y[:].bitcast(u32), 0x7F800000, None,
                mybir.AluOpType.bitwise_and,
            )
            ul = small.tile([1, 1], f32, name=f"{name}_ul")
            nc.vector.tensor_scalar_mul(ul[:], um[:], 2.0 ** -23)
            cand = c3("cand")
            nc.vector.tensor_copy(cand[:, :, 0:1], y[:, :, None])
            tt(cand[:, :, 1:2], y[:, :, None], ul[:, :, None], ADD)
            tt(cand[:, :, 2:3], y[:, :, None], ul[:, :, None], SUB)
            nc.vector.tensor_scalar_mul(cand[:, :, 3:4], y[:, :, None], 1.0 - 2.0 ** -24)
            nc.vector.tensor_scalar_mul(cand[:, :, 4:5], y[:, :, None], 1.0 + 2.0 ** -24)

            dhb = dh[:, :, None].to_broadcast((1, 1, NCAND))
            dlb = dl[:, :, None].to_broadcast((1, 1, NCAND))
            db = d1[:, :, None].to_broadcast((1, 1, NCAND))
            Rc = resid(cand[:], "cc", (1, 1, NCAND), dhb, dlb, db)
            R2c = c3("R2c")
            tt(R2c[:], Rc[:], Rc[:], MUL)
            minr = small.tile([1, 1], f32, name=f"{name}_minr")
            nc.vector.tensor_reduce(minr[:], R2c[:], axis=X, op=mybir.AluOpType.min)
            mask = c3("mask")
            tt(mask[:], R2c[:], minr[:, :, None].to_broadcast((1, 1, NCAND)),
               mybir.AluOpType.is_equal)
            qm = c3("qm")
            tt(qm[:], cand[:], mask[:], MUL)
            s1 = small.tile([1, 1], f32, name=f"{name}_s1")
            nc.vector.tensor_reduce(s1[:], qm[:], axis=X, op=MAX)
            return s1, d1

        last_by_q = {}

        # ---- phase 1a: kernel-shard amax scan (first on the wire) ----
        rk_all = const.tile([P, ko_n], f32, name="rk_all")
        for ko in range(ko_n):
            st = kstage.tile([P, us], f32, tag="kst", name="amx_k_st")
            dma = kscan_qs[ko % 3].dma_start(st[:], ksh[ko * P : (ko + 1) * P, :])
            chain_q(last_by_q, dma, kscan_qs[ko % 3], "k-scan order")
            nc.vector.tensor_reduce(
                rk_all[:, ko : ko + 1], st[:], axis=X, op=MAX,
                apply_absolute_value=True,
            )

        # issue AllReduce(max) for k right away (overlaps the x scan)
        pk1 = small.tile([1, 1], f32, name="pk1")
        partition_amax_to(pk1[:], rk_all, "pk")
        cck_out = allreduce_max_issue(pk1[:], "cck")

        # ---- phase 1b: x-slice amax scan ----
        rx_all = const.tile([P, ko_n], f32, name="rx_all")
        for ko in range(ko_n):
            st = xstage.tile([P, amx_t], f32, tag="xst", name="amx_x_st")
            q = xscan_qs[ko % 3]
            dma = q.dma_start(st[:, :amx_t], xsl[ko * P : (ko + 1) * P, :])
            chain_q(last_by_q, dma, q, "x-scan after k-scan")
            nc.vector.tensor_reduce(
                rx_all[:, ko : ko + 1], st[:, :amx_t], axis=X, op=MAX,
                apply_absolute_value=True,
            )
        px1 = small.tile([1, 1], f32, name="px1")
        partition_amax_to(px1[:], rx_all, "px")
        ccx_out = allreduce_max_issue(px1[:], "ccx")

        # ---- pre-stage block 0 (fully, in a dedicated f32 tile) ----
        xb0 = xb0p.tile([P, ko_n, B0], f32, name="xb0")
        for ko in range(ko_n):
            q = xb0_qs[ko % 3]
            dma = q.dma_start(xb0[:, ko], xT[ko * P : (ko + 1) * P, 0:B0])
            chain_q(last_by_q, dma, q, "xb0 prestage after x-scan")

        # ---- phase 2a: k re-stream starts NOW (scale-independent), while
        # CC_x is still in flight.  Triggers spread over sync/scalar/gpsimd;
        # gpsimd's queue resumes once the k collective has drained. ----
        kst_tiles = []
        for ko in range(ko_n):
            st = kstage.tile([P, us], f32, tag="kst", name="kq_st")
            q = restream_qs[ko % 3]
            dma = q.dma_start(st[:], ksh[ko * P : (ko + 1) * P, :])
            chain_q(last_by_q, dma, q, "restream after xb0")
            kst_tiles.append(st)

        # ---- k scale: read CC_k result (gpsimd queue), exact division ----
        gk = small.tile([1, 8], f32, name="gk")
        nc.gpsimd.dma_start(gk[:], cck_out[:])
        sk, dk = exact_scale(gk, "sk")
        sk_b = bcast_scalar(sk[:], "sk")

        # bias shard, [P, nu]: bias_t[p, ub] = bias[ub*128 + p]
        bias_t = const.tile([P, nu], f32, name="bias_t")
        nc.gpsimd.dma_start(bias_t[:], bsh.rearrange("(o p) -> p o", p=P))

        # ---- phase 2b: quantize kernel shard in ko order on ACT ----
        kq = kqp.tile([P, ko_n, us], fp8, name="kq")
        for ko in range(ko_n):
            nc.scalar.activation(kq[:, ko], kst_tiles[ko][:], COPY, scale=sk_b[:])

        # ---- x scale ----
        gx = small.tile([1, 8], f32, name="gx")
        nc.gpsimd.dma_start(gx[:], ccx_out[:])
        sx, dx = exact_scale(gx, "sx")
        sx_b = bcast_scalar(sx[:], "sx")

        inv1 = small.tile([1, 1], f32, name="inv1")
        nc.vector.tensor_tensor(inv1[:], dk[:], dx[:], MUL)
        nc.vector.tensor_scalar_mul(inv1[:], inv1[:], 1.0 / (FP8_HW_MAX * FP8_HW_MAX))
        inv_b = bcast_scalar(inv1[:], "inv")

        # ---- phase 3: stream x blocks, fp8 DoubleRow matmuls, fused epilogue ----
        gelu = mybir.ActivationFunctionType.Gelu_apprx_tanh
        dr = mybir.MatmulPerfMode.DoubleRow

        def load_and_quant_block(t0, blk):
            """Stream one token block and DVE-fuse quantize into an xq tile."""
            xq = xqp.tile([P, ko_n, blk], fp8, tag="xq", name="xq")
            for ko in range(ko_n):
                st = xstage.tile([P, tblk], f32, tag="xst", name="xq_st")
                q = xload_q(ko)
                dma = q.dma_start(
                    st[:, :blk], xT[ko * P : (ko + 1) * P, t0 : t0 + blk]
                )
                chain_q(last_by_q, dma, q, "x blocks in consumption order")
                nc.vector.tensor_scalar_mul(xq[:, ko], st[:, :blk], sx_b[:])
            return xq

        def epilogue(pt, ub, c0, w, qi):
            ot = outp.tile([P, nfree], f16, tag="ot", name="ot")
            nc.scalar.activation(
                ot[:, :w], pt[:, :w], gelu,
                bias=bias_t[:, ub : ub + 1], scale=inv_b[:],
            )
            q = xload_q(qi)
            dma = q.dma_start(out[ub * P : (ub + 1) * P, c0 : c0 + w], ot[:, :w])
            chain_q(last_by_q, dma, q, "out after x loads")

        t0 = 0
        for tb, blk in enumerate(blocks):
            if tb == 0:
                # Block 0: quantize the pre-staged f32 tile, then kk-OUTER
                # over two halves of 8 psum banks so the accumulations
                # advance at the pace the re-streamed k slabs arrive.
                xq = xqp.tile([P, ko_n, blk], fp8, tag="xq", name="xq0")
                for ko in range(ko_n):
                    nc.vector.tensor_scalar_mul(xq[:, ko], xb0[:, ko], sx_b[:])
                for half in range(2):
                    u0 = half * 8
                    pts = [
                        psum.tile([P, nfree], f32, tag="ps", name=f"ps0_{u0+i}")
                        for i in range(8)
                    ]
                    for kk in range(kk_n):
                        for i in range(8):
                            ub = u0 + i
                            nc.tensor.matmul(
                                pts[i][:, :blk],
                                kq[:, 2 * kk : 2 * kk + 2, ub * P : (ub + 1) * P],
                                xq[:, 2 * kk : 2 * kk + 2, :],
                                start=(kk == 0),
                                stop=(kk == kk_n - 1),
                                perf_mode=dr,
                            )
                    for i in range(8):
                        epilogue(pts[i], u0 + i, t0, blk, u0 + i)
            else:
                tt_n = max(1, blk // nfree)
                w = min(blk, nfree)
                xq = load_and_quant_block(t0, blk)
                for ub in range(nu):
                    pts = [
                        psum.tile([P, nfree], f32, tag="ps", name=f"ps{ti}")
                        for ti in range(tt_n)
                    ]
                    for kk in range(kk_n):
                        lw = kq[:, 2 * kk : 2 * kk + 2, ub * P : (ub + 1) * P]
                        for ti in range(tt_n):
                            nc.tensor.matmul(
                                pts[ti][:, :w],
                                lw,
                                xq[:, 2 * kk : 2 * kk + 2, ti * w : (ti + 1) * w],
                                start=(kk == 0),
                                stop=(kk == kk_n - 1),
                                perf_mode=dr,
                            )
                    for ti in range(tt_n):
                        epilogue(pts[ti], ub, t0 + ti * w, w, ub + ti)
            t0 += blk

    nc.compile()
    return nc


def make_in_maps(x, kern, bias, n_cores=N_CORES):
    tokens, d_in = x.shape
    us = kern.shape[1] // n_cores
    amx_t = tokens // n_cores
    xT = np.ascontiguousarray(x.T)
    in_maps = []
    for c in range(n_cores):
        in_maps.append(
            {
                "xT": xT,
                "xsl": np.ascontiguousarray(xT[:, c * amx_t : (c + 1) * amx_t]),
                "ksh": np.ascontiguousarray(kern[:, c * us : (c + 1) * us]),
                "bsh": np.ascontiguousarray(bias[c * us : (c + 1) * us]),
            }
        )
    return in_maps


_CACHE = {}


def _built():
    if "nc" not in _CACHE:
        _CACHE["nc"] = build()
    return _CACHE["nc"]


def run(x, kern, bias, trace=False, **kwargs):
    """Run on hardware; returns (full_output, BassKernelResults)."""
    nc = _built()
    in_maps = make_in_maps(x, kern, bias)
    res = run_bass_kernel_spmd(
        nc, in_maps, core_ids=list(range(N_CORES)), trace=trace, **kwargs
    )
    shards = [res.results[c]["out"] for c in range(N_CORES)]
    full = np.concatenate(shards, axis=0)  # [units, tokens] fp16
    return full.T.astype(np.float32), res


def kernel(x, kernel, bias):
    out, _ = run(
        np.ascontiguousarray(x, dtype=np.float32),
        np.ascontiguousarray(kernel, dtype=np.float32),
        np.ascontiguousarray(bias, dtype=np.float32),
    )
    return out


# revision 13
# speedup vs baseline: 1.3111x; 1.3111x over previous
"""Trainium2 Bass kernel: fp8-quantized Dense (8192x4096 @ 4096x16384) + bias + tanh-GELU.

Strategy (tensor-parallel over units, 8 cores), v4 "wide prologue":
  - host: transpose x -> xT [d_in, tokens]; shard kernel/bias along units.
  - device per core:
      phase 1: amax scan of the kernel shard spread over FOUR dma trigger
               queues (sync/scalar/tensor/gpsimd) with 6-deep staging so the
               wire stays saturated; AllReduce(max) for k issues the moment
               the k reduce tree completes.  Then the x-slice scan on three
               queues (gpsimd sits behind the k collective), CC_x at its end.
               The first 8 k slabs are retained f32 in the idle xq-pool
               slots so only 24 slabs need re-streaming.
      phase 2: xb0 (block-0 tokens) prestages f32, then the k re-stream
               starts immediately -- while CC_x is still in flight -- so by
               the time the x scale lands most of the kernel shard is
               already in SBUF.  kq quantizes on ACT in ko order (retained
               slabs first, re-streamed as they arrive).
      phase 3: identical to v3: block 0 runs kk-outer over two halves of 8
               psum banks consuming kq slabs as they are produced; later
               blocks are ub-outer/kk-inner.  Epilogue fuses
               gelu_tanh(psum*inv_scale + bias) on ACT, written as fp16.
  - fp8 numerics: unchanged from v3 -- quantize with the correctly rounded
    s = RNE(224/amax) (Newton + candidate selection), TRN fp8e4 matches
    OCP e4m3fn exactly inside [-240, 240], dequant scale amax_x*amax_k/224^2
    restores the reference computation.
  - output is produced transposed ([units, tokens] per core) in fp16; the
    host gathers shards and returns the [tokens, units] f32 view.
"""

import sys

sys.path.insert(0, "/opt/trn_rl_repo")

from contextlib import ExitStack

import numpy as np

import concourse.bacc as bacc
import concourse.tile as tile
from concourse import mybir
from concourse.bass_utils import run_bass_kernel_spmd

P = 128
FP8_HW_MAX = 224.0  # 448/2: keeps hw fp8 values inside TRN's +/-240 range

TOKENS, D_IN, UNITS, N_CORES = 8192, 4096, 16384, 8

B0 = 256  # first token block: fully f32-staged so its loads never stall


def _blocks(tokens, tblk):
    """Token-block schedule: small warmup blocks so PE starts earlier."""
    assert tokens >= 2 * tblk and tblk >= 512
    head = [B0, B0]
    rest = tokens - sum(head)
    assert rest % tblk == 0
    return head + [tblk] * (rest // tblk)


def build(tokens=TOKENS, d_in=D_IN, units=UNITS, n_cores=N_CORES, tblk=512, nfree=512):
    us = units // n_cores
    ko_n = d_in // P          # 128-row f32 slabs along d_in
    kk_n = d_in // (2 * P)    # DoubleRow (256-contraction) steps
    nu = us // P              # 128-unit output blocks
    amx_t = tokens // n_cores # columns of xT this core amax-scans
    blocks = _blocks(tokens, tblk)

    assert d_in % (2 * P) == 0 and us % P == 0
    assert all(b % nfree == 0 or b in (B0, 2 * B0) for b in blocks)
    assert blocks[0] == B0 and nu == 16

    dt = mybir.dt
    f32 = dt.float32
    f16 = dt.float16
    fp8 = dt.float8e4
    X = mybir.AxisListType.X
    MAX = mybir.AluOpType.max
    COPY = mybir.ActivationFunctionType.Copy

    nc = bacc.Bacc("TRN2", target_bir_lowering=False, debug=False, num_devices=n_cores)
    xT = nc.dram_tensor("xT", [d_in, tokens], f32, kind="ExternalInput").ap()
    xsl = nc.dram_tensor("xsl", [d_in, amx_t], f32, kind="ExternalInput").ap()
    ksh = nc.dram_tensor("ksh", [d_in, us], f32, kind="ExternalInput").ap()
    bsh = nc.dram_tensor("bsh", [us], f32, kind="ExternalInput").ap()
    out = nc.dram_tensor("out", [us, tokens], f16, kind="ExternalOutput").ap()

    # Prologue trigger queues.  SP(sync) and Activation(scalar) have HWDGE
    # queues; gpsimd(Pool) can also trigger DMAs and its sequencer is idle
    # during the scan (the collectives only briefly occupy it), adding
    # descriptor-issue parallelism for the latency-bound scan pipeline.
    kscan_qs = (nc.sync, nc.scalar, nc.gpsimd)
    xscan_qs = (nc.sync, nc.scalar, nc.gpsimd)
    xb0_qs = (nc.sync, nc.scalar, nc.gpsimd)
    restream_qs = (nc.sync, nc.scalar, nc.gpsimd)

    def xload_q(i):
        return (nc.sync, nc.scalar)[i % 2]

    from concourse.tile_rust import add_dep_helper

    class PhaseChainer:
        """Order DMA *phases* per queue on the wire: the first DMA a queue
        issues in a new phase depends on the last DMA it issued in the
        previous phase, so a later phase cannot steal bandwidth from an
        earlier one it must not delay.  DMAs within a phase stay unchained
        so each queue ring keeps multiple transfers in flight."""

        def __init__(self):
            self.prev = {}
            self.cur = {}

        def begin_phase(self):
            self.prev = {**self.prev, **self.cur}
            self.cur = {}

        def dma(self, q, d, reason):
            qid = id(q)
            if qid not in self.cur and qid in self.prev:
                add_dep_helper(d.ins, self.prev[qid].ins, sync=True, reason=reason)
            self.cur[qid] = d

    chains = PhaseChainer()

    # Per-partition SBUF budget (~207 KiB usable): kqp 64 + xqp 2x16=32 +
    # xb0p 32 + outp 5 + kstage 7x8=56 + xstage 4x4=16 + const/small ~1.
    with tile.TileContext(nc) as tc, ExitStack() as ctx:
        const = ctx.enter_context(tc.tile_pool(name="const", bufs=1))
        kstage = ctx.enter_context(tc.tile_pool(name="kstage", bufs=7))
        xstage = ctx.enter_context(tc.tile_pool(name="xstage", bufs=4))
        kqp = ctx.enter_context(tc.tile_pool(name="kqp", bufs=1))
        xqp = ctx.enter_context(tc.tile_pool(name="xqp", bufs=2))
        xb0p = ctx.enter_context(tc.tile_pool(name="xb0p", bufs=1))
        outp = ctx.enter_context(tc.tile_pool(name="outp", bufs=5))
        psum = ctx.enter_context(tc.tile_pool(name="psum", bufs=8, space="PSUM"))
        dram = ctx.enter_context(tc.tile_pool(name="dram", bufs=1, space="DRAM"))
        small = ctx.enter_context(tc.tile_pool(name="small", bufs=1))

        from concourse import bass_isa

        def partition_amax_to(dst, racc, name):
            """[P, ko_n] per-partition maxes -> [1,1] scalar in dst (SBUF)."""
            col = small.tile([P, 1], f32, name=f"{name}_col")
            nc.vector.tensor_reduce(col[:], racc[:], axis=X, op=MAX)
            nc.gpsimd.partition_all_reduce(col[:], col[:], P, bass_isa.ReduceOp.max)
            nc.vector.tensor_copy(dst, col[0:1, :])

        def allreduce_max_issue(src11, name):
            """Issue AllReduce(max) of a [1,1] scalar; returns the shared
            dram tile holding the result (read back separately)."""
            cc_in = dram.tile([1, 8], f32, name=f"{name}_in")
            z8 = small.tile([1, 8], f32, name=f"{name}_z8")
            nc.vector.memset(z8[:], 0.0)
            nc.vector.tensor_copy(z8[:, 0:1], src11)
            nc.sync.dma_start(cc_in[:], z8[:])
            cc_out = dram.tile([1, 8], f32, name=f"{name}_out", addr_space="Shared")
            nc.gpsimd.collective_compute(
                "AllReduce", MAX,
                replica_groups=[list(range(n_cores))],
                ins=[cc_in[:].opt()], outs=[cc_out[:].opt()],
            )
            return cc_out

        def bcast_scalar(src11, name):
            """[1,1] SBUF scalar (partition 0) -> [P,1] SBUF broadcast tile."""
            b = const.tile([P, 1], f32, name=f"{name}_b")
            nc.gpsimd.partition_broadcast(b, src11)
            return b

        # Correctly-rounded s = RNE(224/d): the quantize grid must bit-match
        # the reference's RNE(448/d)/2. DVE has no divide, and a 1-2 ulp-off
        # scale shifts the whole fp8 grid (~2.4e-3 rel err). Newton-refine
        # 224*recip(d) with a Dekker-exact residual, then pick among 5
        # float-constructed neighbor candidates the one minimizing |q*d-224|.
        NCAND = 5
        u32 = dt.uint32
        MUL = mybir.AluOpType.mult
        SUB = mybir.AluOpType.subtract
        ADD = mybir.AluOpType.add

        def tt(o, a, bb, op):
            nc.vector.tensor_tensor(o, a, bb, op)

        def exact_scale(g8, name):
            """g8: [1,8] SBUF allreduce result (slot 0 = amax).
            Returns ([1,1] scale s = RNE(224/max(amax,1e-12)), [1,1] d)."""
            d1 = small.tile([1, 1], f32, name=f"{name}_d1")
            nc.vector.tensor_scalar_max(d1[:], g8[:, 0:1], 1e-12)

            def c3(nm):
                return small.tile([1, 1, NCAND], f32, name=f"{name}_{nm}")

            def vsplit(src, pref, shape=(1, 1)):
                t_ = small.tile(list(shape), f32, name=f"{name}_{pref}_t")
                nc.vector.tensor_scalar_mul(t_[:], src, 4097.0)
                a_ = small.tile(list(shape), f32, name=f"{name}_{pref}_a")
                tt(a_[:], t_[:], src, SUB)
                hi = small.tile(list(shape), f32, name=f"{name}_{pref}_hi")
                tt(hi[:], t_[:], a_[:], SUB)
                lo = small.tile(list(shape), f32, name=f"{name}_{pref}_lo")
                tt(lo[:], src, hi[:], SUB)
                return hi, lo

            dh, dl = vsplit(d1[:], "dsp")

            def resid(qap, nm, shape, dhb, dlb, db):
                """exact q*d - 224 via Dekker two-product (f32 ops only)"""
                p_ = small.tile(list(shape), f32, name=f"{name}_{nm}_p")
                tt(p_[:], qap, db, MUL)
                qh, ql = vsplit(qap, f"{nm}_qs", shape)
                w = small.tile(list(shape), f32, name=f"{name}_{nm}_w")
                tt(w[:], qh[:], dhb, MUL)
                tt(w[:], w[:], p_[:], SUB)
                w2 = small.tile(list(shape), f32, name=f"{name}_{nm}_w2")
                tt(w2[:], qh[:], dlb, MUL)
                tt(w[:], w[:], w2[:], ADD)
                tt(w2[:], ql[:], dhb, MUL)
                tt(w[:], w[:], w2[:], ADD)
                tt(w2[:], ql[:], dlb, MUL)
                tt(w[:], w[:], w2[:], ADD)
                nc.vector.tensor_scalar_sub(p_[:], p_[:], FP8_HW_MAX)
                R_ = small.tile(list(shape), f32, name=f"{name}_{nm}_R")
                tt(R_[:], p_[:], w[:], ADD)
                return R_

            r1 = small.tile([1, 1], f32, name=f"{name}_r1")
            nc.vector.reciprocal(r1[:], d1[:])
            y0 = small.tile([1, 1], f32, name=f"{name}_y0")
            nc.vector.tensor_scalar_mul(y0[:], r1[:], FP8_HW_MAX)
            R0 = resid(y0[:], "n0", (1, 1), dh[:], dl[:], d1[:])
            corr = small.tile([1, 1], f32, name=f"{name}_corr")
            tt(corr[:], R0[:], r1[:], MUL)
            y = small.tile([1, 1], f32, name=f"{name}_y")
            tt(y[:], y0[:], corr[:], SUB)

            um = small.tile([1, 1], f32, name=f"{name}_um")
            nc.vector.tensor_scalar(
                um[:].bitcast(u32), y[:].bitcast(u32), 0x7F800000, None,
                mybir.AluOpType.bitwise_and,
            )
            ul = small.tile([1, 1], f32, name=f"{name}_ul")
            nc.vector.tensor_scalar_mul(ul[:], um[:], 2.0 ** -23)
            cand = c3("cand")
            nc.vector.tensor_copy(cand[:, :, 0:1], y[:, :, None])
            tt(cand[:, :, 1:2], y[:, :, None], ul[:, :, None], ADD)
            tt(cand[:, :, 2:3], y[:, :, None], ul[:, :, None], SUB)
            nc.vector.tensor_scalar_mul(cand[:, :, 3:4], y[:, :, None], 1.0 - 2.0 ** -24)
            nc.vector.tensor_scalar_mul(cand[:, :, 4:5], y[:, :, None], 1.0 + 2.0 ** -24)

            dhb = dh[:, :, None].to_broadcast((1, 1, NCAND))
            dlb = dl[:, :, None].to_broadcast((1, 1, NCAND))
            db = d1[:, :, None].to_broadcast((1, 1, NCAND))
            Rc = resid(cand[:], "cc", (1, 1, NCAND), dhb, dlb, db)
            R2c = c3("R2c")
            tt(R2c[:], Rc[:], Rc[:], MUL)
            minr = small.tile([1, 1], f32, name=f"{name}_minr")
            nc.vector.tensor_reduce(minr[:], R2c[:], axis=X, op=mybir.AluOpType.min)
            mask = c3("mask")
            tt(mask[:], R2c[:], minr[:, :, None].to_broadcast((1, 1, NCAND)),
               mybir.AluOpType.is_equal)
            qm = c3("qm")
            tt(qm[:], cand[:], mask[:], MUL)
            s1 = small.tile([1, 1], f32, name=f"{name}_s1")
            nc.vector.tensor_reduce(s1[:], qm[:], axis=X, op=MAX)
            return s1, d1

        # ---- phase 1a: kernel-shard amax scan (first on the wire) ----
        rk_all = const.tile([P, ko_n], f32, name="rk_all")
        chains.begin_phase()
        for ko in range(ko_n):
            st = kstage.tile([P, us], f32, tag="kst", name="amx_k_st")
            dma = kscan_qs[ko % 3].dma_start(st[:], ksh[ko * P : (ko + 1) * P, :])
            chains.dma(kscan_qs[ko % 3], dma, "k-scan order")
            nc.vector.tensor_reduce(
                rk_all[:, ko : ko + 1], st[:], axis=X, op=MAX,
                apply_absolute_value=True,
            )

        # issue AllReduce(max) for k right away (overlaps the x scan)
        pk1 = small.tile([1, 1], f32, name="pk1")
        partition_amax_to(pk1[:], rk_all, "pk")
        cck_out = allreduce_max_issue(pk1[:], "cck")

        # ---- phase 1b: x-slice amax scan ----
        rx_all = const.tile([P, ko_n], f32, name="rx_all")
        chains.begin_phase()
        for ko in range(ko_n):
            st = xstage.tile([P, amx_t], f32, tag="xst", name="amx_x_st")
            q = xscan_qs[ko % 3]
            dma = q.dma_start(st[:, :amx_t], xsl[ko * P : (ko + 1) * P, :])
            chains.dma(q, dma, "x-scan after k-scan")
            nc.vector.tensor_reduce(
                rx_all[:, ko : ko + 1], st[:, :amx_t], axis=X, op=MAX,
                apply_absolute_value=True,
            )
        px1 = small.tile([1, 1], f32, name="px1")
        partition_amax_to(px1[:], rx_all, "px")
        ccx_out = allreduce_max_issue(px1[:], "ccx")

        # ---- pre-stage block 0 (fully, in a dedicated f32 tile) ----
        xb0 = xb0p.tile([P, ko_n, B0], f32, name="xb0")
        chains.begin_phase()
        for ko in range(ko_n):
            q = xb0_qs[ko % 3]
            dma = q.dma_start(xb0[:, ko], xT[ko * P : (ko + 1) * P, 0:B0])
            chains.dma(q, dma, "xb0 prestage after x-scan")

        # ---- phase 2a: k re-stream starts NOW (scale-independent), while
        # CC_x is still in flight.  Triggers spread over sync/scalar/gpsimd;
        # gpsimd's queue resumes once the k collective has drained. ----
        kst_tiles = []
        chains.begin_phase()
        for ko in range(ko_n):
            st = kstage.tile([P, us], f32, tag="kst", name="kq_st")
            q = restream_qs[ko % 3]
            dma = q.dma_start(st[:], ksh[ko * P : (ko + 1) * P, :])
            chains.dma(q, dma, "restream after xb0")
            kst_tiles.append(st)

        # ---- k scale: read CC_k result (gpsimd queue), exact division ----
        gk = small.tile([1, 8], f32, name="gk")
        nc.gpsimd.dma_start(gk[:], cck_out[:])
        sk, dk = exact_scale(gk, "sk")
        sk_b = bcast_scalar(sk[:], "sk")

        # bias shard, [P, nu]: bias_t[p, ub] = bias[ub*128 + p]
        bias_t = const.tile([P, nu], f32, name="bias_t")
        nc.gpsimd.dma_start(bias_t[:], bsh.rearrange("(o p) -> p o", p=P))

        # ---- phase 2b: quantize kernel shard in ko order on ACT ----
        kq = kqp.tile([P, ko_n, us], fp8, name="kq")
        for ko in range(ko_n):
            nc.scalar.activation(kq[:, ko], kst_tiles[ko][:], COPY, scale=sk_b[:])

        # ---- x scale ----
        gx = small.tile([1, 8], f32, name="gx")
        nc.gpsimd.dma_start(gx[:], ccx_out[:])
        sx, dx = exact_scale(gx, "sx")
        sx_b = bcast_scalar(sx[:], "sx")

        inv1 = small.tile([1, 1], f32, name="inv1")
        nc.vector.tensor_tensor(inv1[:], dk[:], dx[:], MUL)
        nc.vector.tensor_scalar_mul(inv1[:], inv1[:], 1.0 / (FP8_HW_MAX * FP8_HW_MAX))
        inv_b = bcast_scalar(inv1[:], "inv")

        # ---- phase 3: stream x blocks, fp8 DoubleRow matmuls, fused epilogue ----
        gelu = mybir.ActivationFunctionType.Gelu_apprx_tanh
        dr = mybir.MatmulPerfMode.DoubleRow

        def load_and_quant_block(t0, blk):
            """Stream one token block and DVE-fuse quantize into an xq tile."""
            xq = xqp.tile([P, ko_n, blk], fp8, tag="xq", name="xq")
            chains.begin_phase()
            for ko in range(ko_n):
                st = xstage.tile([P, tblk], f32, tag="xst", name="xq_st")
                q = xload_q(ko)
                dma = q.dma_start(
                    st[:, :blk], xT[ko * P : (ko + 1) * P, t0 : t0 + blk]
                )
                chains.dma(q, dma, "x blocks in consumption order")
                nc.vector.tensor_scalar_mul(xq[:, ko], st[:, :blk], sx_b[:])
            return xq

        def epilogue(pt, ub, c0, w, qi):
            ot = outp.tile([P, nfree], f16, tag="ot", name="ot")
            nc.scalar.activation(
                ot[:, :w], pt[:, :w], gelu,
                bias=bias_t[:, ub : ub + 1], scale=inv_b[:],
            )
            xload_q(qi).dma_start(out[ub * P : (ub + 1) * P, c0 : c0 + w], ot[:, :w])

        t0 = 0
        for tb, blk in enumerate(blocks):
            if tb == 0:
                # Block 0: quantize the pre-staged f32 tile, then kk-OUTER
                # over two halves of 8 psum banks so the accumulations
                # advance at the pace the re-streamed k slabs arrive.
                xq = xqp.tile([P, ko_n, blk], fp8, tag="xq", name="xq0")
                for ko in range(ko_n):
                    nc.vector.tensor_scalar_mul(xq[:, ko], xb0[:, ko], sx_b[:])
                for half in range(2):
                    u0 = half * 8
                    pts = [
                        psum.tile([P, nfree], f32, tag="ps", name=f"ps0_{u0+i}")
                        for i in range(8)
                    ]
                    for kk in range(kk_n):
                        for i in range(8):
                            ub = u0 + i
                            nc.tensor.matmul(
                                pts[i][:, :blk],
                                kq[:, 2 * kk : 2 * kk + 2, ub * P : (ub + 1) * P],
                                xq[:, 2 * kk : 2 * kk + 2, :],
                                start=(kk == 0),
                                stop=(kk == kk_n - 1),
                                perf_mode=dr,
                            )
                    for i in range(8):
                        epilogue(pts[i], u0 + i, t0, blk, u0 + i)
            else:
                tt_n = max(1, blk // nfree)
                w = min(blk, nfree)
                xq = load_and_quant_block(t0, blk)
                for ub in range(nu):
                    pts = [
                        psum.tile([P, nfree], f32, tag="ps", name=f"ps{ti}")
                        for ti in range(tt_n)
                    ]
                    for kk in range(kk_n):
                        lw = kq[:, 2 * kk : 2 * kk + 2, ub * P : (ub + 1) * P]
                        for ti in range(tt_n):
                            nc.tensor.matmul(
                                pts[ti][:, :w],
                                lw,
                                xq[:, 2 * kk : 2 * kk + 2, ti * w : (ti + 1) * w],
                                start=(kk == 0),
                                stop=(kk == kk_n - 1),
                                perf_mode=dr,
                            )
                    for ti in range(tt_n):
                        epilogue(pts[ti], ub, t0 + ti * w, w, ub + ti)
            t0 += blk

    nc.compile()
    return nc


def make_in_maps(x, kern, bias, n_cores=N_CORES):
    tokens, d_in = x.shape
    us = kern.shape[1] // n_cores
    amx_t = tokens // n_cores
    xT = np.ascontiguousarray(x.T)
    in_maps = []
    for c in range(n_cores):
        in_maps.append(
            {
                "xT": xT,
                "xsl": np.ascontiguousarray(xT[:, c * amx_t : (c + 1) * amx_t]),
                "ksh": np.ascontiguousarray(kern[:, c * us : (c + 1) * us]),
                "bsh": np.ascontiguousarray(bias[c * us : (c + 1) * us]),
            }
        )
    return in_maps


_CACHE = {}


def _built():
    if "nc" not in _CACHE:
        _CACHE["nc"] = build()
    return _CACHE["nc"]


def run(x, kern, bias, trace=False, **kwargs):
    """Run on hardware; returns (full_output, BassKernelResults)."""
    nc = _built()
    in_maps = make_in_maps(x, kern, bias)
    res = run_bass_kernel_spmd(
        nc, in_maps, core_ids=list(range(N_CORES)), trace=trace, **kwargs
    )
    shards = [res.results[c]["out"] for c in range(N_CORES)]
    full = np.concatenate(shards, axis=0)  # [units, tokens] fp16
    return full.T.astype(np.float32), res


def kernel(x, kernel, bias):
    out, _ = run(
        np.ascontiguousarray(x, dtype=np.float32),
        np.ascontiguousarray(kernel, dtype=np.float32),
        np.ascontiguousarray(bias, dtype=np.float32),
    )
    return out


# revision 15
# speedup vs baseline: 1.3433x; 1.0245x over previous
"""Trainium2 Bass kernel: fp8-quantized Dense (8192x4096 @ 4096x16384) + bias + tanh-GELU.

Strategy (tensor-parallel over units, 8 cores), v4 "wide prologue":
  - host: transpose x -> xT [d_in, tokens]; shard kernel/bias along units.
  - device per core:
      phase 1: amax scan of the kernel shard spread over FOUR dma trigger
               queues (sync/scalar/tensor/gpsimd) with 6-deep staging so the
               wire stays saturated; AllReduce(max) for k issues the moment
               the k reduce tree completes.  Then the x-slice scan on three
               queues (gpsimd sits behind the k collective), CC_x at its end.
               The first 8 k slabs are retained f32 in the idle xq-pool
               slots so only 24 slabs need re-streaming.
      phase 2: xb0 (block-0 tokens) prestages f32, then the k re-stream
               starts immediately -- while CC_x is still in flight -- so by
               the time the x scale lands most of the kernel shard is
               already in SBUF.  kq quantizes on ACT in ko order (retained
               slabs first, re-streamed as they arrive).
      phase 3: identical to v3: block 0 runs kk-outer over two halves of 8
               psum banks consuming kq slabs as they are produced; later
               blocks are ub-outer/kk-inner.  Epilogue fuses
               gelu_tanh(psum*inv_scale + bias) on ACT, written as fp16.
  - fp8 numerics: unchanged from v3 -- quantize with the correctly rounded
    s = RNE(224/amax) (Newton + candidate selection), TRN fp8e4 matches
    OCP e4m3fn exactly inside [-240, 240], dequant scale amax_x*amax_k/224^2
    restores the reference computation.
  - output is produced transposed ([units, tokens] per core) in fp16; the
    host gathers shards and returns the [tokens, units] f32 view.
"""

import sys

sys.path.insert(0, "/opt/trn_rl_repo")

from contextlib import ExitStack

import numpy as np

import concourse.bacc as bacc
import concourse.tile as tile
from concourse import mybir
from concourse.bass_utils import run_bass_kernel_spmd

P = 128
FP8_HW_MAX = 224.0  # 448/2: keeps hw fp8 values inside TRN's +/-240 range

TOKENS, D_IN, UNITS, N_CORES = 8192, 4096, 16384, 8

B0 = 256  # first token block: fully f32-staged so its loads never stall


def _blocks(tokens, tblk):
    """Token-block schedule: small warmup blocks so PE starts earlier."""
    assert tokens >= 2 * tblk and tblk >= 512
    head = [B0, B0]
    rest = tokens - sum(head)
    assert rest % tblk == 0
    return head + [tblk] * (rest // tblk)


def build(tokens=TOKENS, d_in=D_IN, units=UNITS, n_cores=N_CORES, tblk=512, nfree=512):
    us = units // n_cores
    ko_n = d_in // P          # 128-row f32 slabs along d_in
    kk_n = d_in // (2 * P)    # DoubleRow (256-contraction) steps
    nu = us // P              # 128-unit output blocks
    amx_t = tokens // n_cores # columns of xT this core amax-scans
    blocks = _blocks(tokens, tblk)

    assert d_in % (2 * P) == 0 and us % P == 0
    assert all(b % nfree == 0 or b in (B0, 2 * B0) for b in blocks)
    assert blocks[0] == B0 and nu == 16

    dt = mybir.dt
    f32 = dt.float32
    f16 = dt.float16
    fp8 = dt.float8e4
    X = mybir.AxisListType.X
    MAX = mybir.AluOpType.max
    COPY = mybir.ActivationFunctionType.Copy

    nc = bacc.Bacc("TRN2", target_bir_lowering=False, debug=False, num_devices=n_cores)
    xT = nc.dram_tensor("xT", [d_in, tokens], f32, kind="ExternalInput").ap()
    xsl = nc.dram_tensor("xsl", [d_in, amx_t], f32, kind="ExternalInput").ap()
    ksh = nc.dram_tensor("ksh", [d_in, us], f32, kind="ExternalInput").ap()
    bsh = nc.dram_tensor("bsh", [us], f32, kind="ExternalInput").ap()
    out = nc.dram_tensor("out", [us, tokens], f16, kind="ExternalOutput").ap()

    # Prologue trigger queues.  SP(sync) and Activation(scalar) have HWDGE
    # queues; gpsimd(Pool) can also trigger DMAs and its sequencer is idle
    # during the scan (the collectives only briefly occupy it), adding
    # descriptor-issue parallelism for the latency-bound scan pipeline.
    kscan_qs = (nc.sync, nc.scalar, nc.gpsimd)
    xscan_qs = (nc.sync, nc.scalar, nc.gpsimd)
    xb0_qs = (nc.sync, nc.scalar, nc.gpsimd)
    restream_qs = (nc.sync, nc.scalar, nc.gpsimd)

    def xload_q(i):
        return (nc.sync, nc.scalar)[i % 2]

    from concourse.tile_rust import add_dep_helper

    class PhaseChainer:
        """Order DMA *phases* per queue on the wire: the first DMA a queue
        issues in a new phase depends on the last DMA it issued in the
        previous phase, so a later phase cannot steal bandwidth from an
        earlier one it must not delay.  DMAs within a phase stay unchained
        so each queue ring keeps multiple transfers in flight."""

        def __init__(self):
            self.prev = {}
            self.cur = {}

        def begin_phase(self):
            self.prev = {**self.prev, **self.cur}
            self.cur = {}

        def dma(self, q, d, reason):
            qid = id(q)
            if qid not in self.cur and qid in self.prev:
                add_dep_helper(d.ins, self.prev[qid].ins, sync=True, reason=reason)
            self.cur[qid] = d

    chains = PhaseChainer()

    # Per-partition SBUF budget (~207 KiB usable): kqp 64 + xqp 2x16=32 +
    # xb0p 32 + outp 5 + kstage 3x16=48 + xstage 3x8=24 + const/small ~1.
    with tile.TileContext(nc) as tc, ExitStack() as ctx:
        const = ctx.enter_context(tc.tile_pool(name="const", bufs=1))
        kstage = ctx.enter_context(tc.tile_pool(name="kstage", bufs=3))
        xstage = ctx.enter_context(tc.tile_pool(name="xstage", bufs=3))
        kqp = ctx.enter_context(tc.tile_pool(name="kqp", bufs=1))
        xqp = ctx.enter_context(tc.tile_pool(name="xqp", bufs=2))
        xb0p = ctx.enter_context(tc.tile_pool(name="xb0p", bufs=1))
        outp = ctx.enter_context(tc.tile_pool(name="outp", bufs=5))
        psum = ctx.enter_context(tc.tile_pool(name="psum", bufs=8, space="PSUM"))
        dram = ctx.enter_context(tc.tile_pool(name="dram", bufs=1, space="DRAM"))
        small = ctx.enter_context(tc.tile_pool(name="small", bufs=1))

        from concourse import bass_isa

        def partition_amax_to(dst, racc, name):
            """[P, ko_n] per-partition maxes -> [1,1] scalar in dst (SBUF)."""
            col = small.tile([P, 1], f32, name=f"{name}_col")
            nc.vector.tensor_reduce(col[:], racc[:], axis=X, op=MAX)
            nc.gpsimd.partition_all_reduce(col[:], col[:], P, bass_isa.ReduceOp.max)
            nc.vector.tensor_copy(dst, col[0:1, :])

        def allreduce_max_issue(src11, name):
            """Issue AllReduce(max) of a [1,1] scalar; returns the shared
            dram tile holding the result (read back separately)."""
            cc_in = dram.tile([1, 8], f32, name=f"{name}_in")
            z8 = small.tile([1, 8], f32, name=f"{name}_z8")
            nc.vector.memset(z8[:], 0.0)
            nc.vector.tensor_copy(z8[:, 0:1], src11)
            nc.sync.dma_start(cc_in[:], z8[:])
            cc_out = dram.tile([1, 8], f32, name=f"{name}_out", addr_space="Shared")
            nc.gpsimd.collective_compute(
                "AllReduce", MAX,
                replica_groups=[list(range(n_cores))],
                ins=[cc_in[:].opt()], outs=[cc_out[:].opt()],
            )
            return cc_out

        def bcast_scalar(src11, name):
            """[1,1] SBUF scalar (partition 0) -> [P,1] SBUF broadcast tile."""
            b = const.tile([P, 1], f32, name=f"{name}_b")
            nc.gpsimd.partition_broadcast(b, src11)
            return b

        # Correctly-rounded s = RNE(224/d): the quantize grid must bit-match
        # the reference's RNE(448/d)/2. DVE has no divide, and a 1-2 ulp-off
        # scale shifts the whole fp8 grid (~2.4e-3 rel err). Newton-refine
        # 224*recip(d) with a Dekker-exact residual, then pick among 5
        # float-constructed neighbor candidates the one minimizing |q*d-224|.
        NCAND = 5
        u32 = dt.uint32
        MUL = mybir.AluOpType.mult
        SUB = mybir.AluOpType.subtract
        ADD = mybir.AluOpType.add

        def tt(o, a, bb, op):
            nc.vector.tensor_tensor(o, a, bb, op)

        def exact_scale(g8, name):
            """g8: [1,8] SBUF allreduce result (slot 0 = amax).
            Returns ([1,1] scale s = RNE(224/max(amax,1e-12)), [1,1] d)."""
            d1 = small.tile([1, 1], f32, name=f"{name}_d1")
            nc.vector.tensor_scalar_max(d1[:], g8[:, 0:1], 1e-12)

            def c3(nm):
                return small.tile([1, 1, NCAND], f32, name=f"{name}_{nm}")

            def vsplit(src, pref, shape=(1, 1)):
                t_ = small.tile(list(shape), f32, name=f"{name}_{pref}_t")
                nc.vector.tensor_scalar_mul(t_[:], src, 4097.0)
                a_ = small.tile(list(shape), f32, name=f"{name}_{pref}_a")
                tt(a_[:], t_[:], src, SUB)
                hi = small.tile(list(shape), f32, name=f"{name}_{pref}_hi")
                tt(hi[:], t_[:], a_[:], SUB)
                lo = small.tile(list(shape), f32, name=f"{name}_{pref}_lo")
                tt(lo[:], src, hi[:], SUB)
                return hi, lo

            dh, dl = vsplit(d1[:], "dsp")

            def resid(qap, nm, shape, dhb, dlb, db):
                """exact q*d - 224 via Dekker two-product (f32 ops only)"""
                p_ = small.tile(list(shape), f32, name=f"{name}_{nm}_p")
                tt(p_[:], qap, db, MUL)
                qh, ql = vsplit(qap, f"{nm}_qs", shape)
                w = small.tile(list(shape), f32, name=f"{name}_{nm}_w")
                tt(w[:], qh[:], dhb, MUL)
                tt(w[:], w[:], p_[:], SUB)
                w2 = small.tile(list(shape), f32, name=f"{name}_{nm}_w2")
                tt(w2[:], qh[:], dlb, MUL)
                tt(w[:], w[:], w2[:], ADD)
                tt(w2[:], ql[:], dhb, MUL)
                tt(w[:], w[:], w2[:], ADD)
                tt(w2[:], ql[:], dlb, MUL)
                tt(w[:], w[:], w2[:], ADD)
                nc.vector.tensor_scalar_sub(p_[:], p_[:], FP8_HW_MAX)
                R_ = small.tile(list(shape), f32, name=f"{name}_{nm}_R")
                tt(R_[:], p_[:], w[:], ADD)
                return R_

            r1 = small.tile([1, 1], f32, name=f"{name}_r1")
            nc.vector.reciprocal(r1[:], d1[:])
            y0 = small.tile([1, 1], f32, name=f"{name}_y0")
            nc.vector.tensor_scalar_mul(y0[:], r1[:], FP8_HW_MAX)
            R0 = resid(y0[:], "n0", (1, 1), dh[:], dl[:], d1[:])
            corr = small.tile([1, 1], f32, name=f"{name}_corr")
            tt(corr[:], R0[:], r1[:], MUL)
            y = small.tile([1, 1], f32, name=f"{name}_y")
            tt(y[:], y0[:], corr[:], SUB)

            um = small.tile([1, 1], f32, name=f"{name}_um")
            nc.vector.tensor_scalar(
                um[:].bitcast(u32), y[:].bitcast(u32), 0x7F800000, None,
                mybir.AluOpType.bitwise_and,
            )
            ul = small.tile([1, 1], f32, name=f"{name}_ul")
            nc.vector.tensor_scalar_mul(ul[:], um[:], 2.0 ** -23)
            cand = c3("cand")
            nc.vector.tensor_copy(cand[:, :, 0:1], y[:, :, None])
            tt(cand[:, :, 1:2], y[:, :, None], ul[:, :, None], ADD)
            tt(cand[:, :, 2:3], y[:, :, None], ul[:, :, None], SUB)
            nc.vector.tensor_scalar_mul(cand[:, :, 3:4], y[:, :, None], 1.0 - 2.0 ** -24)
            nc.vector.tensor_scalar_mul(cand[:, :, 4:5], y[:, :, None], 1.0 + 2.0 ** -24)

            dhb = dh[:, :, None].to_broadcast((1, 1, NCAND))
            dlb = dl[:, :, None].to_broadcast((1, 1, NCAND))
            db = d1[:, :, None].to_broadcast((1, 1, NCAND))
            Rc = resid(cand[:], "cc", (1, 1, NCAND), dhb, dlb, db)
            R2c = c3("R2c")
            tt(R2c[:], Rc[:], Rc[:], MUL)
            minr = small.tile([1, 1], f32, name=f"{name}_minr")
            nc.vector.tensor_reduce(minr[:], R2c[:], axis=X, op=mybir.AluOpType.min)
            mask = c3("mask")
            tt(mask[:], R2c[:], minr[:, :, None].to_broadcast((1, 1, NCAND)),
               mybir.AluOpType.is_equal)
            qm = c3("qm")
            tt(qm[:], cand[:], mask[:], MUL)
            s1 = small.tile([1, 1], f32, name=f"{name}_s1")
            nc.vector.tensor_reduce(s1[:], qm[:], axis=X, op=MAX)
            return s1, d1

        # ---- phase 1a: kernel-shard amax scan (first on the wire) ----
        # Wide 2MB transfers ([P, 2, us] covering two 128-row slabs) halve
        # the DMA ring-entry count: the scan is latency-bound per ring
        # entry, not bandwidth-bound.
        kw = ko_n // 2
        rk_all = const.tile([P, ko_n], f32, name="rk_all")
        chains.begin_phase()
        for i in range(kw):
            st = kstage.tile([P, 2, us], f32, tag="kst", name="amx_k_st")
            src_ap = ksh[i * 2 * P : (i + 1) * 2 * P, :].rearrange(
                "(j p) c -> p j c", p=P
            )
            dma = kscan_qs[i % 3].dma_start(st[:], src_ap)
            chains.dma(kscan_qs[i % 3], dma, "k-scan order")
            nc.vector.tensor_reduce(
                rk_all[:, 2 * i : 2 * i + 2], st[:], axis=X, op=MAX,
                apply_absolute_value=True,
            )

        # issue AllReduce(max) for k right away (overlaps the x scan)
        pk1 = small.tile([1, 1], f32, name="pk1")
        partition_amax_to(pk1[:], rk_all, "pk")
        cck_out = allreduce_max_issue(pk1[:], "cck")

        # ---- phase 1b: x-slice amax scan (wide 1MB transfers) ----
        xw = ko_n // 2
        rx_all = const.tile([P, ko_n], f32, name="rx_all")
        chains.begin_phase()
        for i in range(xw):
            st = xstage.tile([P, 2, amx_t], f32, tag="xst", name="amx_x_st")
            q = xscan_qs[i % 3]
            src_ap = xsl[i * 2 * P : (i + 1) * 2 * P, :].rearrange(
                "(j p) t -> p j t", p=P
            )
            dma = q.dma_start(st[:], src_ap)
            chains.dma(q, dma, "x-scan after k-scan")
            nc.vector.tensor_reduce(
                rx_all[:, 2 * i : 2 * i + 2], st[:], axis=X, op=MAX,
                apply_absolute_value=True,
            )
        px1 = small.tile([1, 1], f32, name="px1")
        partition_amax_to(px1[:], rx_all, "px")
        ccx_out = allreduce_max_issue(px1[:], "ccx")

        # ---- pre-stage block 0 (fully, in a dedicated f32 tile) ----
        xb0 = xb0p.tile([P, ko_n, B0], f32, name="xb0")
        chains.begin_phase()
        for i in range(ko_n // 4):
            q = xb0_qs[i % 3]
            src_ap = xT[i * 4 * P : (i + 1) * 4 * P, 0:B0].rearrange(
                "(j p) t -> p j t", p=P
            )
            dma = q.dma_start(xb0[:, 4 * i : 4 * i + 4], src_ap)
            chains.dma(q, dma, "xb0 prestage after x-scan")

        # ---- phase 2a: k re-stream starts NOW (scale-independent), while
        # CC_x is still in flight.  Triggers spread over sync/scalar/gpsimd;
        # gpsimd's queue resumes once the k collective has drained. ----
        kst_tiles = []
        chains.begin_phase()
        for i in range(kw):
            st = kstage.tile([P, 2, us], f32, tag="kst", name="kq_st")
            q = restream_qs[i % 3]
            src_ap = ksh[i * 2 * P : (i + 1) * 2 * P, :].rearrange(
                "(j p) c -> p j c", p=P
            )
            dma = q.dma_start(st[:], src_ap)
            chains.dma(q, dma, "restream after xb0")
            kst_tiles.append(st)

        # ---- k scale: read CC_k result (gpsimd queue), exact division ----
        gk = small.tile([1, 8], f32, name="gk")
        nc.gpsimd.dma_start(gk[:], cck_out[:])
        sk, dk = exact_scale(gk, "sk")
        sk_b = bcast_scalar(sk[:], "sk")

        # bias shard, [P, nu]: bias_t[p, ub] = bias[ub*128 + p]
        bias_t = const.tile([P, nu], f32, name="bias_t")
        nc.gpsimd.dma_start(bias_t[:], bsh.rearrange("(o p) -> p o", p=P))

        # ---- phase 2b: quantize kernel shard in ko order on ACT ----
        kq = kqp.tile([P, ko_n, us], fp8, name="kq")
        for i in range(kw):
            nc.scalar.activation(
                kq[:, 2 * i : 2 * i + 2], kst_tiles[i][:], COPY, scale=sk_b[:]
            )

        # ---- x scale ----
        gx = small.tile([1, 8], f32, name="gx")
        nc.gpsimd.dma_start(gx[:], ccx_out[:])
        sx, dx = exact_scale(gx, "sx")
        sx_b = bcast_scalar(sx[:], "sx")

        inv1 = small.tile([1, 1], f32, name="inv1")
        nc.vector.tensor_tensor(inv1[:], dk[:], dx[:], MUL)
        nc.vector.tensor_scalar_mul(inv1[:], inv1[:], 1.0 / (FP8_HW_MAX * FP8_HW_MAX))
        inv_b = bcast_scalar(inv1[:], "inv")

        # ---- phase 3: stream x blocks, fp8 DoubleRow matmuls, fused epilogue ----
        gelu = mybir.ActivationFunctionType.Gelu_apprx_tanh
        dr = mybir.MatmulPerfMode.DoubleRow

        def load_and_quant_block(t0, blk):
            """Stream one token block and DVE-fuse quantize into an xq tile."""
            xq = xqp.tile([P, ko_n, blk], fp8, tag="xq", name="xq")
            chains.begin_phase()
            for i in range(ko_n // 2):
                st = xstage.tile([P, 2, amx_t], f32, tag="xst", name="xq_st")
                q = xload_q(i)
                src_ap = xT[i * 2 * P : (i + 1) * 2 * P, t0 : t0 + blk].rearrange(
                    "(j p) t -> p j t", p=P
                )
                dma = q.dma_start(st[:, :, :blk], src_ap)
                chains.dma(q, dma, "x blocks in consumption order")
                nc.vector.tensor_scalar_mul(
                    xq[:, 2 * i : 2 * i + 2], st[:, :, :blk], sx_b[:]
                )
            return xq

        def epilogue(pt, ub, c0, w, qi):
            ot = outp.tile([P, nfree], f16, tag="ot", name="ot")
            nc.scalar.activation(
                ot[:, :w], pt[:, :w], gelu,
                bias=bias_t[:, ub : ub + 1], scale=inv_b[:],
            )
            xload_q(qi).dma_start(out[ub * P : (ub + 1) * P, c0 : c0 + w], ot[:, :w])

        t0 = 0
        for tb, blk in enumerate(blocks):
            if tb == 0:
                # Block 0: quantize the pre-staged f32 tile, then kk-OUTER
                # over two halves of 8 psum banks so the accumulations
                # advance at the pace the re-streamed k slabs arrive.
                xq = xqp.tile([P, ko_n, blk], fp8, tag="xq", name="xq0")
                nc.vector.tensor_scalar_mul(xq[:, :, :], xb0[:, :, :], sx_b[:])
                for half in range(2):
                    u0 = half * 8
                    pts = [
                        psum.tile([P, nfree], f32, tag="ps", name=f"ps0_{u0+i}")
                        for i in range(8)
                    ]
                    for kk in range(kk_n):
                        for i in range(8):
                            ub = u0 + i
                            nc.tensor.matmul(
                                pts[i][:, :blk],
                                kq[:, 2 * kk : 2 * kk + 2, ub * P : (ub + 1) * P],
                                xq[:, 2 * kk : 2 * kk + 2, :],
                                start=(kk == 0),
                                stop=(kk == kk_n - 1),
                                perf_mode=dr,
                            )
                    for i in range(8):
                        epilogue(pts[i], u0 + i, t0, blk, u0 + i)
            else:
                tt_n = max(1, blk // nfree)
                w = min(blk, nfree)
                xq = load_and_quant_block(t0, blk)
                for ub in range(nu):
                    pts = [
                        psum.tile([P, nfree], f32, tag="ps", name=f"ps{ti}")
                        for ti in range(tt_n)
                    ]
                    for kk in range(kk_n):
                        lw = kq[:, 2 * kk : 2 * kk + 2, ub * P : (ub + 1) * P]
                        for ti in range(tt_n):
                            nc.tensor.matmul(
                                pts[ti][:, :w],
                                lw,
                                xq[:, 2 * kk : 2 * kk + 2, ti * w : (ti + 1) * w],
                                start=(kk == 0),
                                stop=(kk == kk_n - 1),
                                perf_mode=dr,
                            )
                    for ti in range(tt_n):
                        epilogue(pts[ti], ub, t0 + ti * w, w, ub + ti)
            t0 += blk

    nc.compile()
    return nc


def make_in_maps(x, kern, bias, n_cores=N_CORES):
    tokens, d_in = x.shape
    us = kern.shape[1] // n_cores
    amx_t = tokens // n_cores
    xT = np.ascontiguousarray(x.T)
    in_maps = []
    for c in range(n_cores):
        in_maps.append(
            {
                "xT": xT,
                "xsl": np.ascontiguousarray(xT[:, c * amx_t : (c + 1) * amx_t]),
                "ksh": np.ascontiguousarray(kern[:, c * us : (c + 1) * us]),
                "bsh": np.ascontiguousarray(bias[c * us : (c + 1) * us]),
            }
        )
    return in_maps


_CACHE = {}


def _built():
    if "nc" not in _CACHE:
        _CACHE["nc"] = build()
    return _CACHE["nc"]


def run(x, kern, bias, trace=False, **kwargs):
    """Run on hardware; returns (full_output, BassKernelResults)."""
    nc = _built()
    in_maps = make_in_maps(x, kern, bias)
    res = run_bass_kernel_spmd(
        nc, in_maps, core_ids=list(range(N_CORES)), trace=trace, **kwargs
    )
    shards = [res.results[c]["out"] for c in range(N_CORES)]
    full = np.concatenate(shards, axis=0)  # [units, tokens] fp16
    return full.T.astype(np.float32), res


def kernel(x, kernel, bias):
    out, _ = run(
        np.ascontiguousarray(x, dtype=np.float32),
        np.ascontiguousarray(kernel, dtype=np.float32),
        np.ascontiguousarray(bias, dtype=np.float32),
    )
    return out
